# revision 32
# baseline (speedup 1.0000x reference)
"""Multi-head causal attention (B=4, T=2048, E=1024, H=16) on 8 TRN2 NeuronCores.

Sharding: core c handles batch b = c//2 and head-group g = c%2 (8 heads = 512
of the 1024 embedding dims). Each core runs an independent single-core kernel:

  QT = (Wq_g @ xq.T)        [512, T]   (d on partitions, 4 strips of 128)
  KT = (Wk_g @ xkv.T)       [512, T]
  V  = (xkv @ Wv_g.T)       [T, 512]   (t on partitions, + ones column -> VE)
  per (tq-chunk c512, head h):
     S.T[tk_blk j, tq] = KT_h[:, j].T @ QT_h[:, c512]   (K=64 matmul)
     P.T = exp(S.T / 8) * causal_mask                    (ScalarE + DVE)
     O.T[65, 512] += [V_h | 1][tk_blk].T @ P.T           (PSUM accumulate)
     O = transpose(O.T); out = O[:, :64] / O[:, 64]      (PE + DVE)

Matmuls are bf16 with fp32 PSUM accumulation; softmax runs unnormalized exp
(scores are O(1) by construction) with the denominator from the appended ones
column.

Pipeline structure (v4):
- QK blocks are emitted in pairs sharing a 2-bank PSUM tile consumed by a
  single exp ACTIVATE; only the 128-wide diagonal sub-blocks get the causal
  mask multiply.
- PV for step s-1 is interleaved between the QK pairs of step s so the
  TensorEngine never waits on the ScalarE exp chain; the divide epilogue for
  s-2 (PE transposes in bf16 + batched reciprocal) opens each step.
- Projections run as 8-matmul units spread across earlier chunks; slice 3's
  Q/K strips are emitted just-in-time inside chunk 3 (which is otherwise
  ScalarE-bound) to balance engine load across chunks.
- PSUM: one shared 3x[128,1024] pool time-shared by QK pairs and projection
  units (chunk 3 has no projections, so pairs get all 3 slots there), 1 bank
  PV accumulator, 1 bank transpose scratch = 8 banks exactly.
- Startup: weight/x tiles are split in half-tiles DMA'd on 4 different engine
  queues in parallel so the first projection matmul starts ~9us in.
"""

import os
import numpy as np
import ml_dtypes

import concourse.bass as bass
import concourse.bacc as bacc
import concourse.mybir as mybir
import concourse.tile as tile
from concourse.bass_utils import run_bass_kernel_spmd
from concourse.masks import make_identity

F32 = mybir.dt.float32
BF16 = mybir.dt.bfloat16
EXP = mybir.ActivationFunctionType.Exp

P = 128  # partitions
D = 64  # head dim
B, T_FULL, E, H_TOT = 4, 2048, 1024, 16
HLOC = 8  # heads per core
DLOC = HLOC * D  # 512: local slice of E
N_CORES = 8


def build(T=T_FULL):
    """Single-core graph; same graph runs SPMD on all 8 cores."""
    assert T % 512 == 0
    TC = T // 512  # tq chunks of 512
    NTB = T // P  # tk blocks of 128
    KCH = E // P  # 8 contraction chunks for projections
    MCH = DLOC // P  # 4 output strips for QT/KT

    nc = bacc.Bacc("TRN2", target_bir_lowering=False, debug=False,
                   num_devices=N_CORES)

    xqT = nc.dram_tensor("xqT", [E, T], BF16, kind="ExternalInput")
    xkvT = nc.dram_tensor("xkvT", [E, T], BF16, kind="ExternalInput")
    wqT = nc.dram_tensor("wqT", [E, DLOC], BF16, kind="ExternalInput")
    wkT = nc.dram_tensor("wkT", [E, DLOC], BF16, kind="ExternalInput")
    wvT = nc.dram_tensor("wvT", [E, DLOC], BF16, kind="ExternalInput")
    out = nc.dram_tensor("out", [T, DLOC], F32, kind="ExternalOutput")

    xq_v = xqT.ap().rearrange("(k p) t -> p k t", p=P)
    xkv_v = xkvT.ap().rearrange("(k p) t -> p k t", p=P)
    wq_v = wqT.ap().rearrange("(k p) d -> p k d", p=P)
    wk_v = wkT.ap().rearrange("(k p) d -> p k d", p=P)
    wv_v = wvT.ap().rearrange("(k p) d -> p k d", p=P)
    out_v = out.ap().rearrange("(c s p) d -> c p s d", p=P, s=4)

    with tile.TileContext(nc) as tc:
        with (
            tc.tile_pool(name="persist", bufs=1) as persist,
            tc.tile_pool(name="xqpool", bufs=2) as xqpool,
            tc.tile_pool(name="xkpool", bufs=2) as xkpool,
            tc.tile_pool(name="ptpool", bufs=16) as ptpool,
            tc.tile_pool(name="otpool", bufs=2) as otpool,
            tc.tile_pool(name="osb", bufs=2) as osb,
            tc.tile_pool(name="rpool", bufs=2) as rpool,
            tc.tile_pool(name="big_ps", bufs=3, space="PSUM") as big_ps,
            tc.tile_pool(name="pv_ps", bufs=1, space="PSUM") as pv_ps,
            tc.tile_pool(name="tp_ps", bufs=1, space="PSUM") as tp_ps,
        ):
            # per-k lookup: wmap[nm][k] / xmap[which][n][k] -> (tile, local_k)
            wmap = {}
            xmap = {"q": {}, "kv": {}}

            def load_pieces(pool, src, width, groups, name):
                """One tile + one DMA per (k-group, engine); returns per-k
                lookup list. Splitting a tile across two DMAs double-issues
                on hardware, so each piece is its own tile."""
                lut = []
                for k0, k1, eng, tag, bufs in groups:
                    t_ = pool.tile([P, k1 - k0, width], BF16, tag=tag,
                                   bufs=bufs, name=f"{name}{tag}")
                    eng.dma_start(t_[:], src[:, k0:k1])
                    for k in range(k0, k1):
                        lut.append((t_, k - k0))
                return lut

            def get_x(which, n, groups=None):
                cache = xmap[which]
                if n not in cache:
                    pool = xqpool if which == "q" else xkpool
                    src = xq_v if which == "q" else xkv_v
                    c0 = 512 * n
                    if groups is None:
                        groups = [(0, 4, nc.gpsimd, "xa", 2),
                                  (4, 8, nc.gpsimd, "xb", 2)]
                    cache[n] = load_pieces(
                        pool, src[:, :, c0 : c0 + 512], 512, groups,
                        f"x{which}{n}")
                return cache[n]

            # Only sync/scalar/gpsimd can issue DMAs (~93GB/s per ring).
            # The critical 2MB (wq + xq0) splits into thirds, one wq piece
            # and one xq0 piece per ring, ordered so k-groups land in the
            # order the Q units consume them. The rest phase-orders behind:
            # K units (wk+xkv0), then V units (wv); slice-1 x rides on
            # scalar/gpsimd; slices 2-3 load lazily on gpsimd (idle
            # mid-kernel).
            wmap["q"] = load_pieces(
                persist, wq_v, DLOC,
                [(0, 3, nc.sync, "wq0", 1), (3, 6, nc.scalar, "wq1", 1),
                 (6, 8, nc.gpsimd, "wq2", 1)], "w")
            get_x("q", 0, [(0, 3, nc.scalar, "x3a", 1),
                           (3, 6, nc.gpsimd, "x3b", 1),
                           (6, 8, nc.sync, "x2", 1)])
            wmap["k"] = load_pieces(
                persist, wk_v, DLOC,
                [(0, 4, nc.sync, "wk0", 1), (4, 8, nc.scalar, "wk1", 1)],
                "w")
            get_x("kv", 0)
            wmap["v"] = load_pieces(
                persist, wv_v, DLOC,
                [(0, 4, nc.sync, "wv0", 1), (4, 8, nc.scalar, "wv1", 1)],
                "w")
            get_x("q", 1, [(0, 4, nc.scalar, "xa", 2),
                           (4, 8, nc.gpsimd, "xb", 2)])
            get_x("kv", 1, [(0, 4, nc.scalar, "xa", 2),
                            (4, 8, nc.gpsimd, "xb", 2)])

            # ---- constants ----
            ident = persist.tile([P, P], BF16, tag="ident")
            make_identity(nc, ident[:])
            # tri2[:, a, :]: upper triangle (keep col >= row), for the two
            # 128-wide diagonal sub-blocks handled per mask op
            tri2 = persist.tile([P, 2, P], BF16, tag="tri2")
            nc.gpsimd.memset(tri2[:], 1.0)
            for a in range(2):
                nc.gpsimd.affine_select(
                    out=tri2[:, a, :],
                    in_=tri2[:, a, :],
                    compare_op=mybir.AluOpType.is_ge,
                    fill=0.0,
                    base=0,
                    pattern=[[1, P]],
                    channel_multiplier=-1,
                )

            QT = persist.tile([P, MCH, T], BF16, tag="QT")
            KT = persist.tile([P, MCH, T], BF16, tag="KT")
            VE = persist.tile([P, NTB, HLOC, D + 1], BF16, tag="VE")
            nc.vector.memset(VE[:, :, :, D : D + 1], 1.0)

            # ---- projection units (8 matmuls + 1 cast each) ----
            def unit_qk(nm, n, m):
                dst = QT if nm == "q" else KT
                xlut = get_x("q" if nm == "q" else "kv", n)
                wlut = wmap[nm]
                ps = big_ps.tile([P, 1024], F32, tag="big", name="pjps")
                for k in range(KCH):
                    wt, wi = wlut[k]
                    xt, xi = xlut[k]
                    nc.tensor.matmul(
                        ps[:, 0:512],
                        wt[:, wi, P * m : P * m + P],
                        xt[:, xi, :],
                        start=(k == 0),
                        stop=(k == KCH - 1),
                    )
                nc.vector.tensor_copy(dst[:, m, 512 * n : 512 * n + 512],
                                      ps[:, 0:512])

            def unit_v(n, r):
                i = 4 * n + r
                xlut = get_x("kv", n)
                wlut = wmap["v"]
                ps = big_ps.tile([P, 1024], F32, tag="big", name="pvps")
                for k in range(KCH):
                    wt, wi = wlut[k]
                    xt, xi = xlut[k]
                    nc.tensor.matmul(
                        ps[:, 0:512],
                        xt[:, xi, P * r : P * r + P],
                        wt[:, wi, :],
                        start=(k == 0),
                        stop=(k == KCH - 1),
                    )
                nc.vector.tensor_copy(
                    VE[:, i, :, 0:D],
                    ps[:, 0:512].rearrange("p (h d) -> p h d", h=HLOC),
                )

            # ---- attention pieces ----
            def emit_qk_pair(c, h, kind, pidx):
                """Emit one QK pair: 2 matmuls -> 1 exp -> optional mask.

                Returns pv entries (pt, j, tile_off, tq_start, width).
                """
                spo, po = h // 2, D * (h % 2)
                q0 = 512 * c
                ps = big_ps.tile([P, 1024], F32, tag="big", name="qps")
                pt = ptpool.tile([P, 1024], BF16, tag="pt", name="pt")
                if kind == "off":
                    j0 = 2 * pidx
                    for t2 in range(2):
                        j = j0 + t2
                        nc.tensor.matmul(
                            ps[:, 512 * t2 : 512 * t2 + 512],
                            KT[po : po + D, spo, P * j : P * j + P],
                            QT[po : po + D, spo, q0 : q0 + 512],
                            start=True,
                            stop=True,
                        )
                    nc.scalar.activation(pt[:], ps[:], EXP, scale=0.125)
                    return [(pt, j0, 0, 0, 512), (pt, j0 + 1, 512, 0, 512)]
                if kind == "dA":
                    j0, j1 = 4 * c, 4 * c + 1
                    nc.tensor.matmul(
                        ps[:, 0:512],
                        KT[po : po + D, spo, P * j0 : P * j0 + P],
                        QT[po : po + D, spo, q0 : q0 + 512],
                        start=True,
                        stop=True,
                    )
                    nc.tensor.matmul(
                        ps[:, 512:896],
                        KT[po : po + D, spo, P * j1 : P * j1 + P],
                        QT[po : po + D, spo, q0 + 128 : q0 + 512],
                        start=True,
                        stop=True,
                    )
                    nc.scalar.activation(pt[:, 0:896], ps[:, 0:896], EXP,
                                         scale=0.125)
                    # mask regions: block j0 cols [0,128) at off 0; block j1
                    # cols [128,256) at off 512+0; deferred so the mask ops
                    # land after the PV cast in DVE queue order
                    mv = pt[:].rearrange("p (a b) -> p a b", a=2)[:, :, 0:P]
                    masks.append(mv)
                    return [(pt, j0, 0, 0, 512), (pt, j1, 512, 128, 384)]
                # dB
                j2, j3 = 4 * c + 2, 4 * c + 3
                nc.tensor.matmul(
                    ps[:, 0:256],
                    KT[po : po + D, spo, P * j2 : P * j2 + P],
                    QT[po : po + D, spo, q0 + 256 : q0 + 512],
                    start=True,
                    stop=True,
                )
                nc.tensor.matmul(
                    ps[:, 256:384],
                    KT[po : po + D, spo, P * j3 : P * j3 + P],
                    QT[po : po + D, spo, q0 + 384 : q0 + 512],
                    start=True,
                    stop=True,
                )
                nc.scalar.activation(pt[:, 0:384], ps[:, 0:384], EXP,
                                     scale=0.125)
                # mask regions: block j2 cols [256,384) at off 0; block j3
                # cols [384,512) at off 256; deferred like dA's
                mv = pt[:].rearrange("p (a b) -> p a b", a=4)[:, 0:2, 0:P]
                masks.append(mv)
                return [(pt, j2, 0, 256, 256), (pt, j3, 256, 384, 128)]

            # ---- per-step state ----
            osb_tiles = {}
            pend_pv = None  # (c, h, entries) awaiting PV in the next step
            pend_ep = None  # (c, h, ot) awaiting transpose+divide epilogue
            masks = []  # deferred diagonal mask multiplies

            def ep_start(c, h, ot):
                tp4 = tp_ps.tile([P, 4, 80], BF16, tag="tp", name="tp4")
                return [c, h, ot, tp4, 0]

            def ep_step(st):
                c, h, ot, tp4, s4 = st
                nc.tensor.transpose(
                    tp4[:, s4, 0 : D + 1],
                    ot[:, P * s4 : P * s4 + P],
                    ident[0 : D + 1, 0 : D + 1],
                )
                st[4] += 1

            def ep_finish(st):
                c, h, ot, tp4, _ = st
                emit_ep_dve(c, h, tp4)

            def emit_epilogue(c, h, ot):
                st = ep_start(c, h, ot)
                while st[4] < 4:
                    ep_step(st)
                ep_finish(st)

            def emit_ep_dve(c, h, tp4):
                r4 = rpool.tile([P, 4], F32, tag="r", name="r4")
                nc.vector.reciprocal(r4[:], tp4[:, :, D])
                oc = osb_tiles[c]
                for s4 in range(4):
                    nc.vector.tensor_scalar_mul(
                        oc[:, s4, D * h : D * h + D],
                        tp4[:, s4, 0:D],
                        r4[:, s4 : s4 + 1],
                    )
                # the last chunk's output streams out per head-pair, with the
                # final two heads as single-head DMAs, shrinking the tail
                if c == TC - 1:
                    if h in (1, 3, 5):
                        lo = P * (h // 2)
                        nc.sync.dma_start(out_v[c][:, :, lo : lo + P],
                                          oc[:, :, lo : lo + P])
                    elif h >= HLOC - 2:
                        lo = D * h
                        nc.sync.dma_start(out_v[c][:, :, lo : lo + D],
                                          oc[:, :, lo : lo + D])
                elif h == HLOC - 1:
                    nc.sync.dma_start(out_v[c], oc[:])

            def emit_pv_all(c, h, entries, units, quota):
                """PV blocks for (c,h) with proj units interleaved; then the
                PSUM->SBUF cast of the PV accumulator."""
                pv = pv_ps.tile([D + 1, 512], F32, tag="pv", name="pv")
                n = len(entries)
                for idx, (pt, j, off, st, w) in enumerate(entries):
                    if idx % 2 == 1 and units and quota:
                        units.pop(0)()
                        quota -= 1
                    nc.tensor.matmul(
                        pv[:, st : st + w],
                        VE[:, j, h, :],
                        pt[:, off : off + w],
                        start=(idx == 0),
                        stop=(idx == n - 1),
                    )
                while units and quota:
                    units.pop(0)()
                    quota -= 1
                ot = otpool.tile([D + 1, 512], BF16, tag="ot", name="ot")
                nc.vector.tensor_copy(ot[:], pv[:])
                return ot

            # ---- prologue: slice-0 projections ----
            for m in range(MCH):
                unit_qk("q", 0, m)
            for m in range(MCH):
                unit_qk("k", 0, m)
            for r in range(4):
                unit_v(0, r)

            # ---- main steps ----
            units = []
            for c in range(TC):
                osb_tiles[c] = osb.tile([P, 4, 512], F32, tag="o",
                                        name=f"osb{c}")
                if c + 1 < TC:
                    if c + 1 < TC - 1:
                        for m in range(MCH):
                            units.append(
                                lambda m=m, n=c + 1: unit_qk("q", n, m))
                            units.append(
                                lambda m=m, n=c + 1: unit_qk("k", n, m))
                        for r in range(4):
                            units.append(lambda r=r, n=c + 1: unit_v(n, r))
                    else:
                        # chunk 2 emits only slice 3's strip-0 and V blocks;
                        # strips 1-3 run just-in-time inside chunk 3, which
                        # is otherwise ScalarE-bound with no proj filler
                        units.append(lambda n=c + 1: unit_qk("q", n, 0))
                        units.append(lambda n=c + 1: unit_qk("k", n, 0))
                        for r in range(4):
                            units.append(lambda r=r, n=c + 1: unit_v(n, r))
                else:
                    for m in range(1, MCH):
                        units.append(lambda m=m, n=c: unit_qk("q", n, m))
                        units.append(lambda m=m, n=c: unit_qk("k", n, m))
                for h in range(HLOC):
                    if c == TC - 1:
                        # strip m is first read at (c, 2m): emit its Q/K
                        # units during steps 2m-2 and 2m-1
                        quota = 1 if h < 6 else 0
                    else:
                        quota = len(units) if h == HLOC - 1 else 2
                    kinds = [("off", p) for p in range(2 * c)]
                    kinds += [("dA", 0), ("dB", 0)]
                    entries = []
                    for kind, pidx in kinds:
                        entries += emit_qk_pair(c, h, kind, pidx)
                        if len(entries) == 4:
                            # the epilogue's transposes wait on the previous
                            # step's PV cast; the pairs (and a proj unit, in
                            # short steps) emitted first hide that latency
                            if units and quota:
                                units.pop(0)()
                                quota -= 1
                            if pend_ep is not None:
                                emit_epilogue(*pend_ep)
                                pend_ep = None
                    if pend_pv is not None:
                        c1, h1, e1 = pend_pv
                        ot = emit_pv_all(c1, h1, e1, units, quota)
                        pend_ep = (c1, h1, ot)
                    # deferred diagonal masks flush after the PV cast so the
                    # cast isn't queue-blocked on DVE behind masks waiting
                    # for this step's diagonal exps
                    for mv in masks:
                        nc.vector.tensor_mul(mv, mv, tri2[:])
                    masks.clear()
                    pend_pv = (c, h, entries)

            # ---- drain ----
            if pend_ep is not None:
                emit_epilogue(*pend_ep)
            c1, h1, e1 = pend_pv
            ot = emit_pv_all(c1, h1, e1, [], 0)
            emit_epilogue(c1, h1, ot)

    nc.compile()
    return nc


_NC_CACHE = {}


def _get_nc(T):
    if T not in _NC_CACHE:
        _NC_CACHE[T] = build(T)
    return _NC_CACHE[T]


def kernel(inputs_q, inputs_kv, Wq, Wk, Wv):
    inputs_q = np.asarray(inputs_q, dtype=np.float32)
    inputs_kv = np.asarray(inputs_kv, dtype=np.float32)
    Wq = np.asarray(Wq, dtype=np.float32)
    Wk = np.asarray(Wk, dtype=np.float32)
    Wv = np.asarray(Wv, dtype=np.float32)
    T = inputs_q.shape[1]

    bf = ml_dtypes.bfloat16
    in_maps = []
    for c in range(N_CORES):
        b, g = c // 2, c % 2
        sl = slice(g * DLOC, (g + 1) * DLOC)
        in_maps.append(
            {
                "xqT": np.ascontiguousarray(inputs_q[b].T).astype(bf),
                "xkvT": np.ascontiguousarray(inputs_kv[b].T).astype(bf),
                "wqT": np.ascontiguousarray(Wq[sl].T).astype(bf),
                "wkT": np.ascontiguousarray(Wk[sl].T).astype(bf),
                "wvT": np.ascontiguousarray(Wv[sl].T).astype(bf),
            }
        )

    nc = _get_nc(T)
    trace = bool(int(os.environ.get("KERNEL_TRACE", "0")))
    res = run_bass_kernel_spmd(
        nc, in_maps, core_ids=list(range(N_CORES)), trace=trace
    )
    if trace:
        kernel.last_result = res

    full = np.empty((B, T, E), np.float32)
    for c in range(N_CORES):
        b, g = c // 2, c % 2
        full[b, :, g * DLOC : (g + 1) * DLOC] = res.results[c]["out"]
    return full


# revision 33
# speedup vs baseline: 1.2109x; 1.2109x over previous
"""Multi-head causal attention (B=4, T=2048, E=1024, H=16) on 8 TRN2 NeuronCores.

Sharding: core c handles batch b = c//2 and head-group g = c%2 (8 heads = 512
of the 1024 embedding dims). Each core runs an independent single-core kernel:

  QT = (Wq_g @ xq.T)        [512, T]   (d on partitions, 4 strips of 128)
  KT = (Wk_g @ xkv.T)       [512, T]
  V  = (xkv @ Wv_g.T)       [T, 512]   (t on partitions, + ones column -> VE)
  per (tq-chunk c512, head h):
     S.T[tk_blk j, tq] = KT_h[:, j].T @ QT_h[:, c512]   (K=64 matmul)
     P.T = exp(S.T / 8) * causal_mask                    (ScalarE + DVE)
     O.T[65, 512] += [V_h | 1][tk_blk].T @ P.T           (PSUM accumulate)
     O = transpose(O.T); out = O[:, :64] / O[:, 64]      (PE + DVE)

Matmuls are bf16 with fp32 PSUM accumulation; softmax runs unnormalized exp
(scores are O(1) by construction) with the denominator from the appended ones
column.

Pipeline structure (v4):
- QK blocks are emitted in pairs sharing a 2-bank PSUM tile consumed by a
  single exp ACTIVATE; only the 128-wide diagonal sub-blocks get the causal
  mask multiply.
- PV for step s-1 is interleaved between the QK pairs of step s so the
  TensorEngine never waits on the ScalarE exp chain; the divide epilogue for
  s-2 (PE transposes in bf16 + batched reciprocal) opens each step.
- Projections run as 8-matmul units spread across earlier chunks; slice 3's
  Q/K strips are emitted just-in-time inside chunk 3 (which is otherwise
  ScalarE-bound) to balance engine load across chunks.
- PSUM: one shared 3x[128,1024] pool time-shared by QK pairs and projection
  units (chunk 3 has no projections, so pairs get all 3 slots there), 1 bank
  PV accumulator, 1 bank transpose scratch = 8 banks exactly.
- Startup: weight/x tiles are split in half-tiles DMA'd on 4 different engine
  queues in parallel so the first projection matmul starts ~9us in.
"""

import os
import numpy as np
import ml_dtypes

import concourse.bass as bass
import concourse.bacc as bacc
import concourse.mybir as mybir
import concourse.tile as tile
from concourse.bass_utils import run_bass_kernel_spmd
from concourse.masks import make_identity

F32 = mybir.dt.float32
BF16 = mybir.dt.bfloat16
EXP = mybir.ActivationFunctionType.Exp

P = 128  # partitions
D = 64  # head dim
B, T_FULL, E, H_TOT = 4, 2048, 1024, 16
HLOC = 8  # heads per core
DLOC = HLOC * D  # 512: local slice of E
N_CORES = 8


def build(T=T_FULL):
    """Single-core graph; same graph runs SPMD on all 8 cores."""
    assert T % 512 == 0
    TC = T // 512  # tq chunks of 512
    NTB = T // P  # tk blocks of 128
    KCH = E // P  # 8 contraction chunks for projections
    MCH = DLOC // P  # 4 output strips for QT/KT

    nc = bacc.Bacc("TRN2", target_bir_lowering=False, debug=False,
                   num_devices=N_CORES)

    xqT = nc.dram_tensor("xqT", [E, T], BF16, kind="ExternalInput")
    xkvT = nc.dram_tensor("xkvT", [E, T], BF16, kind="ExternalInput")
    wqT = nc.dram_tensor("wqT", [E, DLOC], BF16, kind="ExternalInput")
    wkT = nc.dram_tensor("wkT", [E, DLOC], BF16, kind="ExternalInput")
    wvT = nc.dram_tensor("wvT", [E, DLOC], BF16, kind="ExternalInput")
    out = nc.dram_tensor("out", [T, DLOC], F32, kind="ExternalOutput")

    xq_v = xqT.ap().rearrange("(k p) t -> p k t", p=P)
    xkv_v = xkvT.ap().rearrange("(k p) t -> p k t", p=P)
    wq_v = wqT.ap().rearrange("(k p) d -> p k d", p=P)
    wk_v = wkT.ap().rearrange("(k p) d -> p k d", p=P)
    wv_v = wvT.ap().rearrange("(k p) d -> p k d", p=P)
    out_v = out.ap().rearrange("(c s p) d -> c p s d", p=P, s=4)

    with tile.TileContext(nc) as tc:
        with (
            tc.tile_pool(name="persist", bufs=1) as persist,
            tc.tile_pool(name="xqpool", bufs=2) as xqpool,
            tc.tile_pool(name="xkpool", bufs=2) as xkpool,
            tc.tile_pool(name="ptpool", bufs=16) as ptpool,
            tc.tile_pool(name="otpool", bufs=2) as otpool,
            tc.tile_pool(name="osb", bufs=2) as osb,
            tc.tile_pool(name="rpool", bufs=2) as rpool,
            tc.tile_pool(name="big_ps", bufs=3, space="PSUM") as big_ps,
            tc.tile_pool(name="pv_ps", bufs=1, space="PSUM") as pv_ps,
            tc.tile_pool(name="tp_ps", bufs=1, space="PSUM") as tp_ps,
        ):
            wts = {}

            def load_w(nm, src, eng_a, eng_b):
                # two half-tiles on two DMA queues: parallel HBM streams and
                # the first projection matmuls only wait for half a tile
                wa = persist.tile([P, KCH // 2, DLOC], BF16, tag=f"w{nm}a",
                                  name=f"w{nm}a")
                wb = persist.tile([P, KCH // 2, DLOC], BF16, tag=f"w{nm}b",
                                  name=f"w{nm}b")
                eng_a.dma_start(wa[:], src[:, 0:4, :])
                eng_b.dma_start(wb[:], src[:, 4:8, :])
                wts[nm] = (wa, wb)

            x_tiles = {"q": {}, "kv": {}}

            def get_x(which, n, eng_a=None, eng_b=None):
                cache = x_tiles[which]
                if n not in cache:
                    pool = xqpool if which == "q" else xkpool
                    src = xq_v if which == "q" else xkv_v
                    xa = pool.tile([P, KCH // 2, 512], BF16, tag="xa",
                                   name=f"x{which}{n}a")
                    xb = pool.tile([P, KCH // 2, 512], BF16, tag="xb",
                                   name=f"x{which}{n}b")
                    c0 = 512 * n
                    (eng_a or nc.gpsimd).dma_start(
                        xa[:], src[:, 0:4, c0 : c0 + 512])
                    (eng_b or nc.gpsimd).dma_start(
                        xb[:], src[:, 4:8, c0 : c0 + 512])
                    cache[n] = (xa, xb)
                return cache[n]

            # Only sync/scalar/gpsimd can issue DMAs (~93GB/s per ring).
            # Phase-order the three rings to match prologue consumption:
            # Q units (wq+xq0), then K units (wk+xkv0), then V units (wv).
            # Slice-1 x rides behind on scalar/gpsimd; slices 2-3 load
            # lazily on gpsimd, which is idle mid-kernel.
            load_w("q", wq_v, nc.sync, nc.scalar)
            get_x("q", 0, nc.gpsimd, nc.gpsimd)
            load_w("k", wk_v, nc.sync, nc.scalar)
            get_x("kv", 0, nc.gpsimd, nc.gpsimd)
            load_w("v", wv_v, nc.sync, nc.scalar)
            get_x("q", 1, nc.scalar, nc.gpsimd)
            get_x("kv", 1, nc.scalar, nc.gpsimd)

            # ---- constants ----
            ident = persist.tile([P, P], BF16, tag="ident")
            make_identity(nc, ident[:])
            # tri2[:, a, :]: upper triangle (keep col >= row), for the two
            # 128-wide diagonal sub-blocks handled per mask op
            tri2 = persist.tile([P, 2, P], BF16, tag="tri2")
            nc.gpsimd.memset(tri2[:], 1.0)
            for a in range(2):
                nc.gpsimd.affine_select(
                    out=tri2[:, a, :],
                    in_=tri2[:, a, :],
                    compare_op=mybir.AluOpType.is_ge,
                    fill=0.0,
                    base=0,
                    pattern=[[1, P]],
                    channel_multiplier=-1,
                )

            QT = persist.tile([P, MCH, T], BF16, tag="QT")
            KT = persist.tile([P, MCH, T], BF16, tag="KT")
            VE = persist.tile([P, NTB, HLOC, D + 1], BF16, tag="VE")
            nc.vector.memset(VE[:, :, :, D : D + 1], 1.0)

            # ---- projection units (8 matmuls + 1 cast each) ----
            def unit_qk(nm, n, m):
                dst = QT if nm == "q" else KT
                xt = get_x("q" if nm == "q" else "kv", n)
                wt = wts[nm]
                ps = big_ps.tile([P, 1024], F32, tag="big", name="pjps")
                for k in range(KCH):
                    nc.tensor.matmul(
                        ps[:, 0:512],
                        wt[k // 4][:, k % 4, P * m : P * m + P],
                        xt[k // 4][:, k % 4, :],
                        start=(k == 0),
                        stop=(k == KCH - 1),
                    )
                nc.vector.tensor_copy(dst[:, m, 512 * n : 512 * n + 512],
                                      ps[:, 0:512])

            def unit_v(n, r):
                i = 4 * n + r
                xt = get_x("kv", n)
                wt = wts["v"]
                ps = big_ps.tile([P, 1024], F32, tag="big", name="pvps")
                for k in range(KCH):
                    nc.tensor.matmul(
                        ps[:, 0:512],
                        xt[k // 4][:, k % 4, P * r : P * r + P],
                        wt[k // 4][:, k % 4, :],
                        start=(k == 0),
                        stop=(k == KCH - 1),
                    )
                nc.vector.tensor_copy(
                    VE[:, i, :, 0:D],
                    ps[:, 0:512].rearrange("p (h d) -> p h d", h=HLOC),
                )

            # ---- attention pieces ----
            def emit_qk_pair(c, h, kind, pidx):
                """Emit one QK pair: 2 matmuls -> 1 exp -> optional mask.

                Returns pv entries (pt, j, tile_off, tq_start, width).
                """
                spo, po = h // 2, D * (h % 2)
                q0 = 512 * c
                ps = big_ps.tile([P, 1024], F32, tag="big", name="qps")
                pt = ptpool.tile([P, 1024], BF16, tag="pt", name="pt")
                if kind == "off":
                    j0 = 2 * pidx
                    for t2 in range(2):
                        j = j0 + t2
                        nc.tensor.matmul(
                            ps[:, 512 * t2 : 512 * t2 + 512],
                            KT[po : po + D, spo, P * j : P * j + P],
                            QT[po : po + D, spo, q0 : q0 + 512],
                            start=True,
                            stop=True,
                        )
                    nc.scalar.activation(pt[:], ps[:], EXP, scale=0.125)
                    return [(pt, j0, 0, 0, 512), (pt, j0 + 1, 512, 0, 512)]
                if kind == "dA":
                    j0, j1 = 4 * c, 4 * c + 1
                    nc.tensor.matmul(
                        ps[:, 0:512],
                        KT[po : po + D, spo, P * j0 : P * j0 + P],
                        QT[po : po + D, spo, q0 : q0 + 512],
                        start=True,
                        stop=True,
                    )
                    nc.tensor.matmul(
                        ps[:, 512:896],
                        KT[po : po + D, spo, P * j1 : P * j1 + P],
                        QT[po : po + D, spo, q0 + 128 : q0 + 512],
                        start=True,
                        stop=True,
                    )
                    nc.scalar.activation(pt[:, 0:896], ps[:, 0:896], EXP,
                                         scale=0.125)
                    # mask regions: block j0 cols [0,128) at off 0; block j1
                    # cols [128,256) at off 512+0; deferred so the mask ops
                    # land after the PV cast in DVE queue order
                    mv = pt[:].rearrange("p (a b) -> p a b", a=2)[:, :, 0:P]
                    masks.append(mv)
                    return [(pt, j0, 0, 0, 512), (pt, j1, 512, 128, 384)]
                # dB
                j2, j3 = 4 * c + 2, 4 * c + 3
                nc.tensor.matmul(
                    ps[:, 0:256],
                    KT[po : po + D, spo, P * j2 : P * j2 + P],
                    QT[po : po + D, spo, q0 + 256 : q0 + 512],
                    start=True,
                    stop=True,
                )
                nc.tensor.matmul(
                    ps[:, 256:384],
                    KT[po : po + D, spo, P * j3 : P * j3 + P],
                    QT[po : po + D, spo, q0 + 384 : q0 + 512],
                    start=True,
                    stop=True,
                )
                nc.scalar.activation(pt[:, 0:384], ps[:, 0:384], EXP,
                                     scale=0.125)
                # mask regions: block j2 cols [256,384) at off 0; block j3
                # cols [384,512) at off 256; deferred like dA's
                mv = pt[:].rearrange("p (a b) -> p a b", a=4)[:, 0:2, 0:P]
                masks.append(mv)
                return [(pt, j2, 0, 256, 256), (pt, j3, 256, 384, 128)]

            # ---- per-step state ----
            osb_tiles = {}
            pend_pv = None  # (c, h, entries) awaiting PV in the next step
            pend_ep = None  # (c, h, ot) awaiting transpose+divide epilogue
            masks = []  # deferred diagonal mask multiplies

            def ep_start(c, h, ot):
                tp4 = tp_ps.tile([P, 4, 80], BF16, tag="tp", name="tp4")
                return [c, h, ot, tp4, 0]

            def ep_step(st):
                c, h, ot, tp4, s4 = st
                nc.tensor.transpose(
                    tp4[:, s4, 0 : D + 1],
                    ot[:, P * s4 : P * s4 + P],
                    ident[0 : D + 1, 0 : D + 1],
                )
                st[4] += 1

            def ep_finish(st):
                c, h, ot, tp4, _ = st
                emit_ep_dve(c, h, tp4)

            def emit_epilogue(c, h, ot):
                st = ep_start(c, h, ot)
                while st[4] < 4:
                    ep_step(st)
                ep_finish(st)

            def emit_ep_dve(c, h, tp4):
                r4 = rpool.tile([P, 4], F32, tag="r", name="r4")
                nc.vector.reciprocal(r4[:], tp4[:, :, D])
                oc = osb_tiles[c]
                for s4 in range(4):
                    nc.vector.tensor_scalar_mul(
                        oc[:, s4, D * h : D * h + D],
                        tp4[:, s4, 0:D],
                        r4[:, s4 : s4 + 1],
                    )
                # the last chunk's output streams out per head-pair, with the
                # final two heads as single-head DMAs, shrinking the tail
                if c == TC - 1:
                    if h in (1, 3, 5):
                        lo = P * (h // 2)
                        nc.sync.dma_start(out_v[c][:, :, lo : lo + P],
                                          oc[:, :, lo : lo + P])
                    elif h >= HLOC - 2:
                        lo = D * h
                        nc.sync.dma_start(out_v[c][:, :, lo : lo + D],
                                          oc[:, :, lo : lo + D])
                elif h == HLOC - 1:
                    nc.sync.dma_start(out_v[c], oc[:])

            def emit_pv_all(c, h, entries, units, quota):
                """PV blocks for (c,h) with proj units interleaved; then the
                PSUM->SBUF cast of the PV accumulator."""
                pv = pv_ps.tile([D + 1, 512], F32, tag="pv", name="pv")
                n = len(entries)
                for idx, (pt, j, off, st, w) in enumerate(entries):
                    if idx % 2 == 1 and units and quota:
                        units.pop(0)()
                        quota -= 1
                    nc.tensor.matmul(
                        pv[:, st : st + w],
                        VE[:, j, h, :],
                        pt[:, off : off + w],
                        start=(idx == 0),
                        stop=(idx == n - 1),
                    )
                while units and quota:
                    units.pop(0)()
                    quota -= 1
                ot = otpool.tile([D + 1, 512], BF16, tag="ot", name="ot")
                nc.vector.tensor_copy(ot[:], pv[:])
                return ot

            # ---- prologue: slice-0 projections ----
            for m in range(MCH):
                unit_qk("q", 0, m)
            for m in range(MCH):
                unit_qk("k", 0, m)
            for r in range(4):
                unit_v(0, r)

            # ---- main steps ----
            units = []
            for c in range(TC):
                osb_tiles[c] = osb.tile([P, 4, 512], F32, tag="o",
                                        name=f"osb{c}")
                if c + 1 < TC:
                    if c + 1 < TC - 1:
                        for m in range(MCH):
                            units.append(
                                lambda m=m, n=c + 1: unit_qk("q", n, m))
                            units.append(
                                lambda m=m, n=c + 1: unit_qk("k", n, m))
                        for r in range(4):
                            units.append(lambda r=r, n=c + 1: unit_v(n, r))
                    else:
                        # chunk 2 emits only slice 3's strip-0 and V blocks;
                        # strips 1-3 run just-in-time inside chunk 3, which
                        # is otherwise ScalarE-bound with no proj filler
                        units.append(lambda n=c + 1: unit_qk("q", n, 0))
                        units.append(lambda n=c + 1: unit_qk("k", n, 0))
                        for r in range(4):
                            units.append(lambda r=r, n=c + 1: unit_v(n, r))
                else:
                    for m in range(1, MCH):
                        units.append(lambda m=m, n=c: unit_qk("q", n, m))
                        units.append(lambda m=m, n=c: unit_qk("k", n, m))
                for h in range(HLOC):
                    if c == TC - 1:
                        # strip m is first read at (c, 2m): emit its Q/K
                        # units during steps 2m-2 and 2m-1
                        quota = 1 if h < 6 else 0
                    else:
                        quota = len(units) if h == HLOC - 1 else 2
                    kinds = [("off", p) for p in range(2 * c)]
                    kinds += [("dA", 0), ("dB", 0)]
                    entries = []
                    for kind, pidx in kinds:
                        entries += emit_qk_pair(c, h, kind, pidx)
                        if len(entries) == 4:
                            # the epilogue's transposes wait on the previous
                            # step's PV cast; the pairs (and a proj unit, in
                            # short steps) emitted first hide that latency
                            if units and quota:
                                units.pop(0)()
                                quota -= 1
                            if pend_ep is not None:
                                emit_epilogue(*pend_ep)
                                pend_ep = None
                    if pend_pv is not None:
                        c1, h1, e1 = pend_pv
                        ot = emit_pv_all(c1, h1, e1, units, quota)
                        pend_ep = (c1, h1, ot)
                    # deferred diagonal masks flush after the PV cast so the
                    # cast isn't queue-blocked on DVE behind masks waiting
                    # for this step's diagonal exps
                    for mv in masks:
                        nc.vector.tensor_mul(mv, mv, tri2[:])
                    masks.clear()
                    pend_pv = (c, h, entries)

            # ---- drain ----
            if pend_ep is not None:
                emit_epilogue(*pend_ep)
            c1, h1, e1 = pend_pv
            ot = emit_pv_all(c1, h1, e1, [], 0)
            emit_epilogue(c1, h1, ot)

    nc.compile()
    return nc


_NC_CACHE = {}


def _get_nc(T):
    if T not in _NC_CACHE:
        _NC_CACHE[T] = build(T)
    return _NC_CACHE[T]


def kernel(inputs_q, inputs_kv, Wq, Wk, Wv):
    inputs_q = np.asarray(inputs_q, dtype=np.float32)
    inputs_kv = np.asarray(inputs_kv, dtype=np.float32)
    Wq = np.asarray(Wq, dtype=np.float32)
    Wk = np.asarray(Wk, dtype=np.float32)
    Wv = np.asarray(Wv, dtype=np.float32)
    T = inputs_q.shape[1]

    bf = ml_dtypes.bfloat16
    in_maps = []
    for c in range(N_CORES):
        b, g = c // 2, c % 2
        sl = slice(g * DLOC, (g + 1) * DLOC)
        in_maps.append(
            {
                "xqT": np.ascontiguousarray(inputs_q[b].T).astype(bf),
                "xkvT": np.ascontiguousarray(inputs_kv[b].T).astype(bf),
                "wqT": np.ascontiguousarray(Wq[sl].T).astype(bf),
                "wkT": np.ascontiguousarray(Wk[sl].T).astype(bf),
                "wvT": np.ascontiguousarray(Wv[sl].T).astype(bf),
            }
        )

    nc = _get_nc(T)
    trace = bool(int(os.environ.get("KERNEL_TRACE", "0")))
    res = run_bass_kernel_spmd(
        nc, in_maps, core_ids=list(range(N_CORES)), trace=trace
    )
    if trace:
        kernel.last_result = res

    full = np.empty((B, T, E), np.float32)
    for c in range(N_CORES):
        b, g = c // 2, c % 2
        full[b, :, g * DLOC : (g + 1) * DLOC] = res.results[c]["out"]
    return full
